# revision 1
# baseline (speedup 1.0000x reference)
"""DIN-style attention layer on 8 Trainium2 NeuronCores.

Problem: q[B,64], k[B,200,64], v[B,200,64], mask[B,200]; per-token MLP on
DIN features concat([q,k,q-k,q*k]) -> 80 -> 40 -> 1 logits, masked softmax
over T, then attn-weighted sum of v. B=2048 sharded over 8 cores (pure
data parallel, batch split).

Math refactor (host):
  info@W1 = q@(W1a+W1c) + k@(W1b-W1c) + (q*k)@W1d   with W1=[W1a;W1b;W1c;W1d]
  => h1_b = relu( Wb_eff^T kt_b + beta_b ),  Wb_eff = (W1b-W1c) + q_b*W1d
     beta_b = q_b@(W1a+W1c) + b1   (folded in as a 65th all-ones row of kt)
bf is dropped: softmax is shift-invariant.

Mask compaction (host): only unmasked t's are shipped/computed. Batches are
sorted by n_b = sum(mask_b) and packed in pairs-of-8 with a per-pair column
capacity cap_j = max n_b of the pair (shared across cores; the Bass build is
cached per caps-tuple). Pad columns are all-zero kt (incl. the ones row) and
zero v, so a pad's logit is exactly Wf^T relu(b2); the host subtracts
n_pad * exp(logit_pad) from the device exp-sum instead of masking on device.
Batches with n_b > 128 (none for the fixed seed) fall back to exact numpy.

Device (per core, 256 batches = 4 DMA groups of 64 = 32 pairs j of 8):
  batch order in pair: q8 = 4*rh + 2*ch + jb
  L1 per chunk (j,rh):  PH1[80, 4cap] (1 PSUM bank) <- 4 matmuls (ch,jb)
  h1-relu -> H1S bf16   (split DVE/ACT by H1_ACT_OF_16 for engine balance)
  L2: PH2[128, 4cap] rows 64*rh <- w2p^T @ H1S (tile_position packs 2 chunks)
  h2-relu+b2 -> H2S bf16 (DVE tensor_scalar add+max)
  L3 transposed: per (j,c4) matmul lhsT=H2S-block, rhs=Wf2col -> PLT[t',4,2]
  (fuses the final layer with the attention transpose: no PE transposes, no
  full-width exp, no PSUM->SBUF copy of attention weights)
  exp (ACT, one [cap,16] op per couple of pairs) -> EXT bf16;
  out = v^T @ attn^T per batch-pair (v stationary, N=2); exp-sums via a
  ones-column matmul into the same OPT bank; one DVE copy each to SBUF
  accumulators, DMA out per group.

Scheduling: engines execute their FIFOs in order, so ops are emitted
software-pipelined in quad-of-chunks slots with per-stage offsets (Q_*),
sized so no engine head-blocks on a not-yet-ready dependency. PSUM banks:
PH1 x4, PH2 x2, PLT x1, OPT x1. Group-0's kwt DMA is split per pair to
shorten the startup ramp. Cost-model timeline: ~45.0us (baseline 125.6us).
"""

import os
import sys

import numpy as np

for _p in ("/opt/trn_rl_repo", "/root/.axon_site/_ro/trn_rl_repo"):
    if os.path.isdir(_p) and _p not in sys.path:
        sys.path.insert(0, _p)

import ml_dtypes

BF16 = ml_dtypes.bfloat16

B, T, D = 2048, 200, 64
H1, H2 = 80, 40
NCORES = 8
BC = B // NCORES          # 256 batches per core
TC = 128                  # compacted (unmasked) tokens per batch, padded
GB = 64                   # batches per device group
NG = BC // GB             # 8 groups
PAIRS = BC // 2           # 128

# engine balance knobs: of every 16 h1-relu ops, this many go to ACT
H1_ACT_OF_16 = 12
# h2-relu ops (one per 4 pairs): how many of every 16 go to ACT
H2_ACT_OF_16 = 0
# PSUM pool buffer counts (8 banks total)
PH1_BUFS, PH2_BUFS, PLT_BUFS, OP_BUFS = 4, 2, 1, 1
# software-pipeline stage offsets (in chunk slots)
Q_R1, Q_L2, Q_H2, Q_PLT, Q_EXO, Q_CP = 1, 4, 5, 6, 7, 8
ABLATE = 0  # 0=full,1=L1+relu,2=+L2,3=+h2relu,4=+plt,5=+expout
DMA_PREF = 2
KW_SPLIT_GROUPS = 1
V_SPLIT = 4
KW_BUFS, V_BUFS = 4, 4
SPLIT_RELU = False
# within-group phases (h1_idx % 16) whose h1-relu runs on DVE
H1_DVE_PHASES = (0, 1, 6, 7, 10)
# within-group phases (h2_idx % 8) whose h2-relu runs on ACT
H2_ACT_PHASES = ()
H1P_BUFS, H2P_BUFS, EX_BUFS = 24, 6, 10


def _build_bass(caps):
    caps = list(caps)
    from concourse import bass, bacc, tile
    from concourse import mybir

    dt = mybir.dt
    nc = bacc.Bacc("TRN2", target_bir_lowering=False, debug=False)

    # kwt packs kt (cols 0:cap) and w1b (cols cap:cap+80) per batch, groups
    # tightly concatenated; v2d is [128, NG*2048] with group g at rows 0:cap_g
    # caps: one entry per batch-pair-of-8 (j); kwt packs 8 batches per pair
    kwoff = [0]
    for cj in caps:
        kwoff.append(kwoff[-1] + 8 * (cj + H1))
    PPG = GB // 8  # pairs per DMA group
    gcap = [max(caps[g * PPG : (g + 1) * PPG]) for g in range(NG)]
    kwt = nc.declare_dram_parameter("kwt", [D + 1, kwoff[-1]], dt.bfloat16, False)
    v2d = nc.declare_dram_parameter("v2d", [TC, NG * GB * 64], dt.bfloat16, False)
    # consts: cols 0:64 w2p (rows 0:80), 64:66 Wf 2-col, 66 ones
    cst = nc.declare_dram_parameter("cst", [128, 68], dt.bfloat16, False)
    b2s = nc.declare_dram_parameter("b2s", [128, 1], dt.float32, False)
    # [:, g, 0:GB//2, :] = v-weighted sums; [0, g, GB//2:GB, :] = exp-sums
    outp = nc.declare_dram_parameter("outp", [128, NG, GB, 2], dt.float32, True)

    W = 4 * TC  # 512 free-dim of all the wide tiles

    h1_idx = 0
    h2_idx = 0

    with tile.TileContext(nc) as tc:
        with (
            tc.tile_pool(name="consts", bufs=1) as cpool,
            tc.tile_pool(name="kwin", bufs=KW_BUFS) as kwpool,
            tc.tile_pool(name="vin", bufs=V_BUFS) as vpool,
            tc.tile_pool(name="h1", bufs=H1P_BUFS) as h1pool,
            tc.tile_pool(name="h2", bufs=H2P_BUFS) as h2pool,
            tc.tile_pool(name="ex", bufs=EX_BUFS) as expool,
            tc.tile_pool(name="ph1", bufs=PH1_BUFS, space="PSUM") as ph1pool,
            tc.tile_pool(name="ph2", bufs=PH2_BUFS, space="PSUM") as ph2pool,
            tc.tile_pool(name="plt", bufs=PLT_BUFS, space="PSUM") as pltpool,
            tc.tile_pool(name="op", bufs=OP_BUFS, space="PSUM") as oppool,
        ):

            outacc = cpool.tile([128, NG, GB, 2], dt.float32)

            # ---- software-pipelined emission ----
            # engines execute their FIFOs in program order, so ops are
            # emitted in (approximate) execution order: slot k emits
            # L1(k), h1relu(k-1), L2(k-2), h2relu/PLT for earlier chunks,
            # and the per-group tail a few slots after the group's chunks.
            NCH = BC // 4  # chunk count; 4 batches per chunk
            KWs, V2s, PH1s, H1Ss, PH2s, PLTs, OPTs, EXTs = {}, {}, {}, {}, {}, {}, {}, {}
            consts = {}

            def dma_group(g):
                o0, o1 = kwoff[g * PPG], kwoff[(g + 1) * PPG]
                KW = kwpool.tile([D + 1, o1 - o0], dt.bfloat16, name="KW")
                if g < KW_SPLIT_GROUPS:
                    # split the load per pair so the first chunks of the
                    # group can start after ~1/8th of the transfer
                    for j in range(g * PPG, (g + 1) * PPG):
                        nc.sync.dma_start(
                            KW[:, kwoff[j] - o0 : kwoff[j + 1] - o0],
                            kwt[:, kwoff[j] : kwoff[j + 1]],
                        )
                else:
                    nc.sync.dma_start(KW[:], kwt[:, o0:o1])
                KWs[g] = KW

            def dma_v(g):
                # v is only needed by the out-matmuls, several slots after
                # the group's kt/weights: issue it after the NEXT group's KW
                # so KW transfers aren't delayed behind it; split in halves
                # so the first pairs' v lands earlier
                V2 = vpool.tile([gcap[g], GB // 2, 128], dt.bfloat16, name="V2")
                h = GB * 32
                for hh in range(V_SPLIT):
                    w = h * 2 // V_SPLIT
                    nc.sync.dma_start(
                        V2[:, hh * (GB // (2 * V_SPLIT)) : (hh + 1) * (GB // (2 * V_SPLIT)), :],
                        v2d[0 : gcap[g], g * GB * 64 + hh * w : g * GB * 64 + (hh + 1) * w],
                    )
                V2s[g] = V2

            def l1(c):
                j, rh = c // 2, c % 2
                g = c // (GB // 4)
                cj = caps[j]
                PH1 = ph1pool.tile([H1, 4 * cj], dt.float32, name="PH1")
                PH1s[c] = PH1
                KW = KWs[g]
                pbase = kwoff[j] - kwoff[g * PPG]
                for c4 in range(4):
                    bo = pbase + (4 * rh + c4) * (cj + H1)
                    nc.tensor.matmul(
                        PH1[:, c4 * cj : (c4 + 1) * cj],
                        lhsT=KW[:, bo + cj : bo + cj + H1],
                        rhs=KW[:, bo : bo + cj],
                        start=True,
                        stop=True,
                    )

            def h1relu(c):
                nonlocal h1_idx
                cg = caps[c // 2]
                H1S = h1pool.tile([H1, 4 * cg], dt.bfloat16, name="H1S")
                H1Ss[c] = H1S
                PH1 = PH1s.pop(c)
                if SPLIT_RELU:
                    h = W // 2
                    nc.scalar.activation(
                        H1S[:, 0:h], PH1[:, 0:h], mybir.ActivationFunctionType.Relu
                    )
                    nc.vector.tensor_scalar_max(H1S[:, h:W], PH1[:, h:W], 0.0)
                elif (
                    ((h1_idx % 16) not in H1_DVE_PHASES)
                    if H1_DVE_PHASES is not None
                    else (h1_idx + 1) * H1_ACT_OF_16 // 16
                    > h1_idx * H1_ACT_OF_16 // 16
                ):
                    nc.scalar.activation(
                        H1S[:], PH1[:], mybir.ActivationFunctionType.Relu
                    )
                else:
                    nc.vector.tensor_scalar_max(H1S[:], PH1[:], 0.0)
                h1_idx += 1

            def l2(c):
                j, rh = c // 2, c % 2
                cg = caps[j]
                if rh == 0:
                    PH2s[j] = ph2pool.tile([128, 4 * cg], dt.float32, name="PH2")
                nc.tensor.matmul(
                    PH2s[j][64 * rh : 64 * rh + 64, :],
                    lhsT=consts["w2p"],
                    rhs=H1Ss.pop(c)[:],
                    start=True,
                    stop=True,
                    tile_position=(0, 64 * rh),
                )

            def h2relu(j):
                nonlocal h2_idx
                cg = caps[j]
                H2S = h2pool.tile([128, 4 * cg], dt.bfloat16, name="H2S")
                PH2 = PH2s.pop(j)
                if SPLIT_RELU:
                    h = W // 2
                    nc.scalar.activation(
                        H2S[:, 0:h],
                        PH2[:, 0:h],
                        mybir.ActivationFunctionType.Relu,
                        bias=consts["b2s"][:],
                    )
                    nc.vector.tensor_scalar(
                        H2S[:, h:W],
                        PH2[:, h:W],
                        consts["b2s"][:],
                        0.0,
                        op0=mybir.AluOpType.add,
                        op1=mybir.AluOpType.max,
                    )
                elif (h2_idx % 8) in H2_ACT_PHASES:
                    nc.scalar.activation(
                        H2S[:],
                        PH2[:],
                        mybir.ActivationFunctionType.Relu,
                        bias=consts["b2s"][:],
                    )
                else:
                    nc.vector.tensor_scalar(
                        H2S[:],
                        PH2[:],
                        consts["b2s"][:],
                        0.0,
                        op0=mybir.AluOpType.add,
                        op1=mybir.AluOpType.max,
                    )
                h2_idx += 1
                H1Ss[("h2", j)] = H2S

            def plt_mms(j):
                cg = caps[j]
                if j % 2 == 0:
                    PLTs[j // 2] = pltpool.tile(
                        [128, 2, 4, 2], dt.float32, name="PLT"
                    )
                PLT = PLTs[j // 2]
                H2S = H1Ss.pop(("h2", j))
                for c4 in range(4):
                    nc.tensor.matmul(
                        PLT[0:cg, j % 2, c4, :],
                        lhsT=H2S[:, c4 * cg : (c4 + 1) * cg],
                        rhs=consts["wf2"],
                        start=True,
                        stop=True,
                    )

            def exp_j(j):
                # one exp covers pairs (j-1, j); wider free dim amortizes the
                # fixed ACT access cost
                cm = max(caps[j - 1], caps[j])
                EXT = expool.tile([128, 2, 4, 2], dt.bfloat16, name="EXT")
                nc.scalar.activation(
                    EXT[0:cm, :, :, :],
                    PLTs.pop(j // 2)[0:cm, :, :, :],
                    mybir.ActivationFunctionType.Exp,
                )
                EXTs[j // 2] = EXT

            def out_mms(j):
                g, jj = j // (GB // 8), j % (GB // 8)
                if jj == 0:
                    OPTs[g] = oppool.tile([128, GB, 2], dt.float32, name="OPT")
                OPT = OPTs[g]
                cg = caps[j]
                EXT = EXTs[j // 2]
                if j % 2 == 1:
                    del EXTs[j // 2]
                V2 = V2s[g]
                for rr in range(4):
                    rh, ch = rr // 2, rr % 2
                    pr = 4 * jj + 2 * rh + ch
                    nc.tensor.matmul(
                        OPT[:, pr, :],
                        lhsT=V2[0:cg, pr, :],
                        rhs=EXT[0:cg, j % 2, 2 * ch : 2 * ch + 2, rh],
                        start=True,
                        stop=True,
                    )
                nc.tensor.matmul(
                    OPT[0:1, GB // 2 + 4 * jj : GB // 2 + 4 * jj + 4, :],
                    lhsT=consts["ones"][0:cg, :],
                    rhs=EXT[0:cg, j % 2, :, :],
                    start=True,
                    stop=True,
                )

            def tail_copy(g):
                V2s.pop(g)
                OPT = OPTs.pop(g)
                nc.vector.tensor_copy(outacc[:, g, :, :], OPT[:])
                nc.sync.dma_start(outp[:, g, :, :], outacc[:, g, :, :])

            # consts load first (small), then the split group-0 kwt
            cst_t = cpool.tile([128, 68], dt.bfloat16)
            nc.sync.dma_start(cst_t[:], cst[:])
            b2s_t = cpool.tile([128, 1], dt.float32)
            nc.sync.dma_start(b2s_t[:], b2s[:])
            # Warm the DVE vector clock past the const DMAs: TensorScalarPtr
            # (relu with AP scalar) only has one sync-wait slot, so it must
            # not be the first DVE op to observe the b2s DMA completion.
            dve_warm = cpool.tile([128, 1], dt.float32)
            nc.vector.tensor_copy(dve_warm[:], b2s_t[:])
            w2p = cst_t[0:H1, 0:64]
            wf2 = cst_t[:, 64:66]
            ones_col = cst_t[:, 66:67]
            consts["w2p"] = w2p
            consts["wf2"] = wf2
            consts["ones"] = ones_col
            consts["b2s"] = b2s_t
            for gg in range(min(DMA_PREF, NG)):
                dma_group(gg)
            dma_v(0)
            NQ = NCH // 4  # quads of 4 chunks
            for s in range(NQ + Q_CP + 3):
                QPG = GB // 16
                if s % QPG == 0 and s < NQ:
                    if s // QPG + DMA_PREF < NG:
                        dma_group(s // QPG + DMA_PREF)
                    if 0 < s // QPG + 1 < NG:
                        dma_v(s // QPG + 1)
                if s < NQ:
                    for c in range(4 * s, 4 * s + 4):
                        l1(c)
                if 0 <= s - Q_R1 < NQ:
                    for c in range(4 * (s - Q_R1), 4 * (s - Q_R1) + 4):
                        h1relu(c)
                if 0 <= s - Q_L2 < NQ and (ABLATE == 0 or ABLATE >= 2):
                    for c in range(4 * (s - Q_L2), 4 * (s - Q_L2) + 4):
                        l2(c)
                if 0 <= s - Q_H2 < NQ and (ABLATE == 0 or ABLATE >= 3):
                    for j in range(2 * (s - Q_H2), 2 * (s - Q_H2) + 2):
                        h2relu(j)
                if 0 <= s - Q_PLT < NQ and (ABLATE == 0 or ABLATE >= 4):
                    for j in range(2 * (s - Q_PLT), 2 * (s - Q_PLT) + 2):
                        plt_mms(j)
                if 0 <= s - Q_EXO < NQ and (ABLATE == 0 or ABLATE >= 5):
                    exp_j(2 * (s - Q_EXO) + 1)
                    for j in range(2 * (s - Q_EXO), 2 * (s - Q_EXO) + 2):
                        out_mms(j)
                QPG2 = GB // 16
                if (
                    s - Q_CP >= 0
                    and (s - Q_CP) % QPG2 == QPG2 - 1
                    and (s - Q_CP) // QPG2 < NG
                    and ABLATE == 0
                ):
                    tail_copy((s - Q_CP) // QPG2)




    nc.compile()
    return nc


_NC_CACHE = {}


def _get_nc(caps=(TC,) * NG):
    key = tuple(caps)
    if key not in _NC_CACHE:
        _NC_CACHE[key] = _build_bass(key)
    return _NC_CACHE[key]


def _prep_core(qc, kc, vc, mc, W1, b1, W2, b2, Wf, caps, perm, n_b):
    """Build the per-core DRAM input dict (numpy, host-side). Batches are
    permuted into ascending-n_b order (perm) and each group g packs its
    batches to caps[g] columns."""
    f32 = np.float32
    W1a, W1b_, W1c, W1d = W1[0:64], W1[64:128], W1[128:192], W1[192:256]

    keep = mc != 0  # [BC, T]
    overflow = np.nonzero(n_b > TC)[0]

    # order[b, :n_b] = indices of kept t's; pads point at t=0 (zeroed later)
    order = np.argsort(~keep, axis=1, kind="stable")[:, :TC]  # [BC, TC]
    colpad = np.arange(TC)[None, :] >= np.minimum(n_b, TC)[:, None]  # [BC, TC]

    bidx = np.arange(BC)[:, None]
    ktp = kc[bidx, order, :]  # [BC, TC, 64]
    vtp = vc[bidx, order, :].astype(f32)  # [BC, TC, 64]
    ktp[colpad] = 0.0
    vtp[colpad] = 0.0
    ones_row = np.where(colpad, f32(1.0), f32(1.0))
    ones_row[colpad] = 0.0

    wb_eff = (W1b_ - W1c)[None, :, :] + qc[:, :, None] * W1d[None, :, :]
    beta = qc @ (W1a + W1c) + b1[None, :]

    kwoff = [0]
    for cj in caps:
        kwoff.append(kwoff[-1] + 8 * (cj + H1))
    gcap = [max(caps[g * (GB // 8) : (g + 1) * (GB // 8)]) for g in range(NG)]
    kwt = np.zeros((D + 1, kwoff[-1]), dtype=BF16)
    v2d = np.zeros((TC, NG * GB * 64), dtype=BF16)
    for j in range(BC // 8):
        cj = caps[j]
        sl = perm[j * 8 : (j + 1) * 8]  # original batch ids, sorted order
        blk = np.empty((D + 1, 8, cj + H1), dtype=BF16)
        blk[0:D, :, 0:cj] = ktp[sl, 0:cj, :].transpose(2, 0, 1).astype(BF16)
        blk[D, :, 0:cj] = ones_row[sl, 0:cj].astype(BF16)
        blk[0:D, :, cj:] = wb_eff[sl].transpose(1, 0, 2).astype(BF16)
        blk[D, :, cj:] = beta[sl].astype(BF16)
        kwt[:, kwoff[j] : kwoff[j + 1]] = blk.reshape(D + 1, 8 * (cj + H1))
    for g in range(NG):
        cg = gcap[g]
        sl = perm[g * GB : (g + 1) * GB]
        # v block [cg, GB//2, 128]: pair pr=4jj+2rh+ch, col jb*64+d;
        # sorted batch slot = 8jj+4rh+2ch+jb
        v6 = vtp[sl, 0:cg, :].reshape(GB // 8, 2, 2, 2, cg, D)  # jj rh ch jb t d
        v2d[0:cg, g * GB * 64 : (g + 1) * GB * 64] = (
            v6.transpose(4, 0, 1, 2, 3, 5).reshape(cg, GB * 64).astype(BF16)
        )

    cst = np.zeros((128, 68), dtype=BF16)
    cst[0:H1, 0:H2] = W2.astype(BF16)
    cst[0:H2, 64] = Wf[:, 0].astype(BF16)
    cst[64 : 64 + H2, 65] = Wf[:, 0].astype(BF16)
    cst[:, 66] = BF16(1.0)
    b2sx = np.zeros((128, 1), dtype=f32)
    b2sx[0:H2, 0] = b2
    b2sx[64 : 64 + H2, 0] = b2

    meta = {"n_b": n_b, "overflow": overflow, "caps": caps, "perm": perm}
    return {"kwt": kwt, "v2d": v2d, "cst": cst, "b2s": b2sx}, meta


def _postprocess(res_c, meta, b2, Wf):
    """outp [128,PAIRS,2] + osum [NG,16,2] -> [BC, D] normalized."""
    raw = np.asarray(res_c["outp"], dtype=np.float32)  # [128, NG, GB, 2]
    op = raw[:, :, 0 : GB // 2, :]
    os_ = raw[0, :, GB // 2 : GB, :].reshape(NG, GB)
    n_b, caps, perm = meta["n_b"], meta["caps"], meta["perm"]

    sb = np.arange(BC)  # sorted (device) batch slot
    g, r = sb // GB, sb % GB
    jj, rem = r // 8, r % 8
    rh, ch, jb = rem // 4, (rem % 4) // 2, rem % 2
    pg = 4 * jj + 2 * rh + ch  # pair within group

    # sums matmul free order (jj, c4=2ch+jb, rh)
    s_raw = os_[g, 8 * jj + 2 * (2 * ch + jb) + rh]  # [BC]
    # device pad columns produce exactly logit = Wf^T relu(b2)
    logit_pad = float(Wf[:, 0] @ np.maximum(b2, 0.0))
    caps_arr = np.asarray(caps)[sb // 8]
    nb_dev = np.minimum(n_b[perm], TC)
    n_pad = np.maximum(caps_arr - nb_dev, 0)
    s = s_raw - n_pad * np.exp(np.float32(logit_pad))
    s = np.where(s <= 1e-30, np.float32(1.0), s).astype(np.float32)

    oc = np.empty((BC, D), dtype=np.float32)
    d = np.arange(D)
    dev = (
        op[(64 * jb)[:, None] + d[None, :], g[:, None], pg[:, None], jb[:, None]]
        / s[:, None]
    )
    oc[perm] = dev
    return oc


def _host_reference_rows(qc, kc, vc, mc, W1, b1, W2, b2, Wf, bf, rows):
    """Exact numpy fallback for overflow batches (n_b > TC)."""
    NEG = np.float32(-(2.0**32) + 1.0)
    out = np.empty((len(rows), D), dtype=np.float32)
    for i, b in enumerate(rows):
        qt = np.broadcast_to(qc[b][None, :], kc[b].shape)
        info = np.concatenate([qt, kc[b], qt - kc[b], qt * kc[b]], axis=-1)
        h = np.maximum(info @ W1 + b1, 0.0)
        h = np.maximum(h @ W2 + b2, 0.0)
        logits = (h @ Wf)[:, 0] + bf[0]
        logits = np.where(mc[b] == 0, NEG, logits)
        e = np.exp(logits - logits.max())
        a = e / e.sum()
        out[i] = a @ vc[b]
    return out


LAST_RESULTS = None
LAST_NC = None


def kernel(q, k, v, mask, W1, b1, W2, b2, Wf, bf, **_):
    from concourse.bass_utils import run_bass_kernel_spmd

    q = np.asarray(q, dtype=np.float32)
    k = np.asarray(k, dtype=np.float32)
    v = np.asarray(v, dtype=np.float32)
    mask = np.asarray(mask)
    W1 = np.asarray(W1, dtype=np.float32)
    b1 = np.asarray(b1, dtype=np.float32)
    W2 = np.asarray(W2, dtype=np.float32)
    b2 = np.asarray(b2, dtype=np.float32)
    Wf = np.asarray(Wf, dtype=np.float32)
    bf = np.asarray(bf, dtype=np.float32)

    # per-core ascending-n_b sort; caps shared across cores (one program).
    # Within each DMA group, pairs are ordered descending so the chain that
    # drains at each group boundary runs on the narrowest pair.
    npair = BC // 8
    ppg = GB // 8
    order = np.concatenate(
        [
            np.arange(g * ppg, (g + 1) * ppg)[:: (-1 if g == NG - 1 else 1)]
            for g in range(NG)
        ]
    )
    nbs, perms = [], []
    for c in range(NCORES):
        mc = mask[c * BC : (c + 1) * BC]
        n_b = (mc != 0).sum(axis=1).astype(np.int64)
        perm = np.argsort(n_b, kind="stable")
        nbs.append(n_b)
        perms.append(perm)
    caps = []
    for j in range(BC // 8):
        cj = max(
            int(np.minimum(nbs[c][perms[c][(j + 1) * 8 - 1]], TC))
            for c in range(NCORES)
        )
        caps.append(max(cj, 8))
    caps = tuple(caps)

    nc = _get_nc(caps)
    global LAST_NC
    LAST_NC = nc
    in_maps, metas = [], []
    for c in range(NCORES):
        s = slice(c * BC, (c + 1) * BC)
        im, meta = _prep_core(
            q[s], k[s], v[s], mask[s], W1, b1, W2, b2, Wf, caps, perms[c], nbs[c]
        )
        in_maps.append(im)
        metas.append(meta)

    trace = bool(int(os.environ.get("BASS_KERNEL_TRACE", "0")))
    tdir = os.environ.get("BASS_KERNEL_TRACE_DIR") or None
    res = run_bass_kernel_spmd(
        nc, in_maps, list(range(NCORES)), trace=trace, tmpdir=tdir
    )
    global LAST_RESULTS
    LAST_RESULTS = res
    results = res.results

    out = np.empty((B, D), dtype=np.float32)
    for c in range(NCORES):
        s = slice(c * BC, (c + 1) * BC)
        out[s] = _postprocess(results[c], metas[c], b2, Wf)
        ov = metas[c]["overflow"]
        if len(ov):
            out[s][ov] = _host_reference_rows(
                q[s], k[s], v[s], mask[s], W1, b1, W2, b2, Wf, bf, ov
            )
    return out


if __name__ == "__main__":
    rng = np.random.default_rng(0)
    inputs = {
        "q": rng.standard_normal((B, D), dtype=np.float32),
        "k": rng.standard_normal((B, T, D), dtype=np.float32),
        "v": rng.standard_normal((B, T, D), dtype=np.float32),
        "mask": rng.integers(0, 2, size=(B, T)).astype(np.int32),
        "W1": rng.standard_normal((4 * D, H1), dtype=np.float32) * 0.05,
        "b1": np.zeros(H1, np.float32),
        "W2": rng.standard_normal((H1, H2), dtype=np.float32) * 0.05,
        "b2": np.zeros(H2, np.float32),
        "Wf": rng.standard_normal((H2, 1), dtype=np.float32) * 0.05,
        "bf": np.zeros(1, np.float32),
    }
    out = kernel(**inputs)
    print(out.shape, out.dtype, np.abs(out).max())

